# revision 10
# baseline (speedup 1.0000x reference)
"""Sliding-window GQA attention (B=2, S=2048, E=4096, HQ=32, HKV=8, D=128,
WINDOW=1024) — full-input / full-output Trainium2 Bass kernel.

Sharding: 8 cores = (batch b in {0,1}) x (4 head groups: 8 q heads / 2 kv
heads each).  Each core runs the same NEFF (SPMD) on its slice:

  inputs  : xT [E,S] bf16 (x[b] transposed), Wq/Wk/Wv slices, Wo slice,
            rope cos/sin tables
  output  : partial out [S,E] bf16 (this head-group's contribution)

Host sums the 4 head-group partials per batch (fp32).

Per-core pipeline (all matmuls bf16, fp32 PSUM accumulation):
  A) q/k/v projections from xT chunks (lhsT = xT e-chunk, moving = weights),
     l2norm + rope on q/k in natural [i,d] layout, PE-transpose q/k to
     [d,i] layout (qT/kT).  v stays natural [j,d] with a ones column
     appended (gives softmax denominators for free in the PV matmul).
  B) per (i-chunk 512, q head): scores sT[j,i] = kT.T @ qT on PE;
     tanh soft-cap (ACT), causal+window masks (DVE affine_select),
     fixed-bias exp (ACT, exp(50*tanh - 50); logits are bounded so no
     row-max pass is needed); PV: out[i, d+1] = probs.T @ [v | 1] with
     probs as the stationary operand; normalize by the ones-column sum;
     PE-transpose to attnT [d,i].
  C) output projection out[i,e] = attnT.T @ Wo, accumulated over the 8
     local heads; bf16 partial stored to HBM.
"""

import os

import numpy as np

try:  # ml_dtypes ships with the env; needed for bf16 host arrays
    import ml_dtypes

    _BF16 = ml_dtypes.bfloat16
except Exception:  # pragma: no cover
    _BF16 = None

import concourse.bass as bass
import concourse.mybir as mybir
import concourse.tile as tile
from concourse.bass_utils import run_bass_kernel_spmd
from concourse.masks import make_identity
from concourse.vector_clock import ScopedClock

# ----------------------------------------------------------------------------
# Workaround: this walrus build supports only ONE semaphore wait per
# instruction (setupSyncWait "Too many sync wait commands").  After tracing,
# split every instruction carrying N>1 waits into (N-1) preceding same-engine
# NOPs with one wait each.
def _split_multi_waits(nc, max_waits=1):
    for f in nc.m.functions:
        for blk in f.blocks:
            il = blk.instructions
            out = []
            changed = False
            for inst in il:
                si = inst.sync_info
                if si is not None and si.on_wait and len(si.on_wait) > max_waits:
                    waits = list(si.on_wait)
                    n_extra = len(waits) - max_waits
                    for i in range(0, n_extra, max_waits):
                        nop = mybir.InstNoOp(
                            name=f"{inst.name}-w{i}",
                            engine=inst.engine,
                            bass_nofuse=True,
                            sync_info=mybir.SyncInfo(
                                on_wait=waits[i : i + max_waits], on_update=[]
                            ),
                        )
                        out.append(nop)
                    si.on_wait = waits[n_extra:]
                    changed = True
                out.append(inst)
            if changed:
                blk.instructions = out

# ----------------------------------------------------------------------------

B, S, E = 2, 2048, 4096
HQ, HKV, D = 32, 8, 128
WINDOW = 1024
SOFT_CAP = 50.0
Q_PRE_ATTN = 128.0
EPS = 1e-6
ROPE_BASE = 10000.0

N_CORES = 8
GROUPS = N_CORES // B          # 4 head groups
HQL = HQ // GROUPS             # 8 q heads per core
HKVL = HKV // GROUPS           # 2 kv heads per core

FULL_CFG = dict(S=S, E=E, HQL=HQL, HKVL=HKVL, W=WINDOW)

F32 = mybir.dt.float32
BF16 = mybir.dt.bfloat16


def _build_nc(cfg=None):
    cfg = cfg or FULL_CFG
    S_, E_, HQL_, HKVL_, W_ = (
        cfg["S"],
        cfg["E"],
        cfg["HQL"],
        cfg["HKVL"],
        cfg["W"],
    )
    Dh = D // 2
    NB = S_ // 128            # seq blocks
    NEC = E_ // 128           # e chunks
    ICH = min(512, S_)        # scores i-chunk width
    NCH = S_ // ICH           # number of i-chunks
    IBC = ICH // 128          # i-blocks per chunk
    QW = min(256, S_)         # phase-A xT piece width (seq cols)
    NP = S_ // QW             # number of pieces
    IBP = QW // 128           # i-blocks per piece
    GL = HQL_ // HKVL_        # local q heads per kv head
    NECH = E_ // 512          # out-proj e'-chunks
    ISQ = float(Q_PRE_ATTN ** -0.5)

    nc = bass.Bass(trn_type="TRN2")

    xT_d = nc.dram_tensor("xT", [E_, S_], BF16, kind="ExternalInput")
    wq_d = nc.dram_tensor("wq", [E_, HQL_ * D], BF16, kind="ExternalInput")
    wk_d = nc.dram_tensor("wk", [E_, HKVL_ * D], BF16, kind="ExternalInput")
    wv_d = nc.dram_tensor("wv", [E_, HKVL_ * D], BF16, kind="ExternalInput")
    wo_d = nc.dram_tensor("wo", [HQL_ * D, E_], BF16, kind="ExternalInput")
    cos_d = nc.dram_tensor("cosb", [S_, Dh], F32, kind="ExternalInput")
    sin_d = nc.dram_tensor("sinb", [S_, Dh], F32, kind="ExternalInput")
    out_d = nc.dram_tensor("out", [S_, E_], BF16, kind="ExternalOutput")

    xr = xT_d.rearrange("(ec p) s -> p ec s", p=128)
    wqr = wq_d.rearrange("(ec p) m -> p ec m", p=128)
    wkr = wk_d.rearrange("(ec p) m -> p ec m", p=128)
    wvr = wv_d.rearrange("(ec p) m -> p ec m", p=128)
    wor = wo_d.rearrange("(h p) e -> p h e", p=128)
    cosr = cos_d.rearrange("(ib p) h -> p ib h", p=128)
    sinr = sin_d.rearrange("(ib p) h -> p ib h", p=128)
    outr = out_d.rearrange("(ib p) e -> ib p e", p=128)

    with tile.TileContext(nc) as tc, tc.tile_pool(name="singles", bufs=1) as singles:
        # ---------------- persistent buffers ----------------
        ident = singles.tile([128, 128], BF16, tag="ident")
        make_identity(nc, ident)
        eps_t = singles.tile([128, 1], F32, tag="eps")
        nc.vector.memset(eps_t, EPS)
        negcap_t = singles.tile([128, 1], F32, tag="negcap")
        nc.vector.memset(negcap_t, -SOFT_CAP)
        cos_sb = singles.tile([128, NB, Dh], F32, tag="cos")
        sin_sb = singles.tile([128, NB, Dh], F32, tag="sin")
        nc.sync.dma_start(cos_sb, cosr)
        nc.sync.dma_start(sin_sb, sinr)
        fill_reg = nc.gpsimd.to_reg(-30.0)
        qT = singles.tile([128, HQL_, S_], BF16, tag="qT")
        kT = singles.tile([128, HKVL_, S_], BF16, tag="kT")
        v_sb = singles.tile([128, HKVL_, NB, D + 1], BF16, tag="v")
        nc.vector.memset(v_sb[:, :, :, D : D + 1], 1.0)

        # ---------------- phase A: projections + norm + rope ------------
        with (
            tc.tile_pool(name="aw", bufs=1) as aw,
            tc.tile_pool(name="ax", bufs=2) as ax,
            tc.tile_pool(name="atmp", bufs=2) as atmp,
            tc.tile_pool(name="astat", bufs=4) as astat,
            tc.tile_pool(name="apsum", bufs=1, space=bass.MemorySpace.PSUM) as aps,
            tc.tile_pool(name="atr", bufs=2, space=bass.MemorySpace.PSUM) as atr,
        ):
            wq_sb = aw.tile([128, NEC, HQL_ * D], BF16, tag="wq")
            wk_sb = aw.tile([128, NEC, HKVL_ * D], BF16, tag="wk")
            wv_sb = aw.tile([128, NEC, HKVL_ * D], BF16, tag="wv")
            nc.sync.dma_start(wq_sb, wqr)
            nc.sync.dma_start(wk_sb, wkr)
            nc.sync.dma_start(wv_sb, wvr)

            def norm_rope(psrc, h, ib, dst, extra_scale):
                """l2norm + rope one head from psum, write bf16 [d,i] into dst."""
                sq = atmp.tile([128, D], F32, tag="sq")
                ssq = astat.tile([128, 1], F32, tag="ssq")
                nc.scalar.activation(
                    sq,
                    psrc[:, h * D : (h + 1) * D],
                    mybir.ActivationFunctionType.Square,
                    accum_out=ssq,
                )
                rstd = astat.tile([128, 1], F32, tag="rstd")
                nc.scalar.activation(
                    rstd,
                    ssq,
                    mybir.ActivationFunctionType.Sqrt,
                    bias=eps_t,
                    scale=1.0 / D,
                )
                rstd2 = astat.tile([128, 1], F32, tag="rstd2")
                nc.vector.reciprocal(rstd2, rstd)
                if extra_scale != 1.0:
                    nc.vector.tensor_scalar_mul(rstd2, rstd2, extra_scale)
                qs = atmp.tile([128, D], F32, tag="qs")
                nc.vector.tensor_scalar_mul(qs, psrc[:, h * D : (h + 1) * D], rstd2)
                cs = cos_sb[:, ib, :]
                sn = sin_sb[:, ib, :]
                tcs = atmp.tile([128, D], F32, tag="tcs")
                tsn = atmp.tile([128, D], F32, tag="tsn")
                nc.vector.tensor_mul(tcs[:, 0:Dh], qs[:, 0:Dh], cs)
                nc.vector.tensor_mul(tcs[:, Dh:D], qs[:, Dh:D], cs)
                nc.vector.tensor_mul(tsn[:, 0:Dh], qs[:, Dh:D], sn)
                nc.vector.tensor_mul(tsn[:, Dh:D], qs[:, 0:Dh], sn)
                qro = atmp.tile([128, D], BF16, tag="qro")
                nc.vector.tensor_sub(qro[:, 0:Dh], tcs[:, 0:Dh], tsn[:, 0:Dh])
                nc.vector.tensor_add(qro[:, Dh:D], tcs[:, Dh:D], tsn[:, Dh:D])
                ptr = atr.tile([128, 128], BF16, tag="ptr")
                nc.tensor.transpose(ptr, qro, ident)
                nc.vector.tensor_copy(dst[:, h, ib * 128 : (ib + 1) * 128], ptr)

            for piece in range(NP):
                xq = ax.tile([128, NEC, QW], BF16, tag="xq")
                nc.sync.dma_start(xq, xr[:, :, piece * QW : (piece + 1) * QW])
                for ibl in range(IBP):
                    ib = piece * IBP + ibl
                    pq = aps.tile([128, HQL_ * D], F32, tag="pq")
                    pk = aps.tile([128, HKVL_ * D], F32, tag="pk")
                    pv = aps.tile([128, HKVL_ * D], F32, tag="pv")
                    for ec in range(NEC):
                        lx = xq[:, ec, ibl * 128 : (ibl + 1) * 128]
                        st, sp = ec == 0, ec == NEC - 1
                        for m0 in range(0, HQL_ * D, 512):
                            m1 = min(m0 + 512, HQL_ * D)
                            nc.tensor.matmul(
                                pq[:, m0:m1],
                                lx,
                                wq_sb[:, ec, m0:m1],
                                start=st,
                                stop=sp,
                            )
                        nc.tensor.matmul(
                            pk, lx, wk_sb[:, ec, :], start=st, stop=sp
                        )
                        nc.tensor.matmul(
                            pv, lx, wv_sb[:, ec, :], start=st, stop=sp
                        )
                    for hk in range(HKVL_):
                        nc.vector.tensor_copy(
                            v_sb[:, hk, ib, 0:D], pv[:, hk * D : (hk + 1) * D]
                        )
                        norm_rope(pk, hk, ib, kT, 1.0)
                    for h in range(HQL_):
                        norm_rope(pq, h, ib, qT, ISQ)

        # ---------------- phases B+C: attention + output projection -----
        with (
            tc.tile_pool(name="bw", bufs=1) as bw,
            tc.tile_pool(name="bprobs", bufs=2) as bprobs,
            tc.tile_pool(name="btmp", bufs=3) as btmp,
            tc.tile_pool(name="bstat", bufs=4) as bstat,
            tc.tile_pool(name="bout", bufs=3) as bout,
            tc.tile_pool(name="bmm", bufs=3, space=bass.MemorySpace.PSUM) as bmm,
            tc.tile_pool(name="bpv", bufs=4, space=bass.MemorySpace.PSUM) as bpv,
            tc.tile_pool(name="btr", bufs=1, space=bass.MemorySpace.PSUM) as btr,
        ):
            wo_sb = bw.tile([128, HQL_, E_], BF16, tag="wo")
            nc.sync.dma_start(wo_sb, wor)
            attnT = bw.tile([128, HQL_, S_], BF16, tag="attnT")

            WB = W_ // 128
            for c in range(NCH):
                jlo = max(0, (c * ICH - (W_ - 1)) // 128)
                jhi = c * IBC + IBC - 1
                njb = jhi - jlo + 1
                for h in range(HQL_):
                    hk = h // GL
                    probs = bprobs.tile([128, njb, ICH], BF16, tag="probs")
                    for jj, jb in enumerate(range(jlo, jhi + 1)):
                        ps = bmm.tile([128, ICH], F32, tag="mm512")
                        nc.tensor.matmul(
                            ps,
                            kT[:, hk, jb * 128 : (jb + 1) * 128],
                            qT[:, h, c * ICH : (c + 1) * ICH],
                        )
                        t = btmp.tile([128, ICH], F32, tag="tanh")
                        nc.scalar.activation(
                            t, ps, mybir.ActivationFunctionType.Tanh,
                            scale=1.0 / SOFT_CAP,
                        )
                        delta = c * ICH - jb * 128
                        if delta < 128:  # causal boundary in tile
                            nc.gpsimd.affine_select(
                                out=t,
                                in_=t,
                                base=delta,
                                channel_multiplier=-1,
                                pattern=[[1, ICH]],
                                compare_op=mybir.AluOpType.is_ge,
                                fill=fill_reg,
                            )
                        if delta + ICH - 1 > W_ - 1:  # window boundary in tile
                            nc.gpsimd.affine_select(
                                out=t,
                                in_=t,
                                base=(W_ - 1) - delta,
                                channel_multiplier=1,
                                pattern=[[-1, ICH]],
                                compare_op=mybir.AluOpType.is_ge,
                                fill=fill_reg,
                            )
                        nc.scalar.activation(
                            probs[:, jj, :], t,
                            mybir.ActivationFunctionType.Exp,
                            bias=negcap_t, scale=SOFT_CAP,
                        )
                    for ibl in range(IBC):
                        ib = c * IBC + ibl
                        jbs = list(range(max(0, (ib * 128 - (W_ - 1)) // 128), ib + 1))
                        pvp = bpv.tile([128, D + 1], F32, tag="pv")
                        for idx, jb in enumerate(jbs):
                            nc.tensor.matmul(
                                pvp,
                                probs[:, jb - jlo, ibl * 128 : (ibl + 1) * 128],
                                v_sb[:, hk, jb, :],
                                start=idx == 0,
                                stop=idx == len(jbs) - 1,
                            )
                        rec = bstat.tile([128, 1], F32, tag="rec")
                        nc.vector.reciprocal(rec, pvp[:, D : D + 1])
                        an = btmp.tile([128, D], BF16, tag="an")
                        nc.vector.tensor_scalar_mul(an, pvp[:, 0:D], rec)
                        ptr = btr.tile([128, 128], BF16, tag="btr")
                        nc.tensor.transpose(ptr, an, ident)
                        nc.vector.tensor_copy(
                            attnT[:, h, ib * 128 : (ib + 1) * 128], ptr
                        )
                # output projection for this chunk's i-blocks
                for ibl in range(IBC):
                    ib = c * IBC + ibl
                    for ech in range(NECH):
                        po = bmm.tile([128, 512], F32, tag="mm512")
                        for h in range(HQL_):
                            nc.tensor.matmul(
                                po,
                                attnT[:, h, ib * 128 : (ib + 1) * 128],
                                wo_sb[:, h, ech * 512 : (ech + 1) * 512],
                                start=h == 0,
                                stop=h == HQL_ - 1,
                            )
                        ot = bout.tile([128, 512], BF16, tag="ot")
                        nc.vector.tensor_copy(ot, po)
                        nc.sync.dma_start(
                            outr[ib, :, ech * 512 : (ech + 1) * 512], ot
                        )
    _split_multi_waits(nc)
    return nc


# ----------------------------------------------------------------------------
# Host side

_NC_CACHE = {}
LAST_RESULTS = None
LAST_EXEC_NS = None


def _get_nc():
    if "nc" not in _NC_CACHE:
        _NC_CACHE["nc"] = _build_nc()
    return _NC_CACHE["nc"]


def _get_runner():
    """Jitted 8-core SPMD executable for the cached nc (mirrors
    bass2jax.run_bass_via_pjrt, but reusable so repeat calls don't
    retrace/recompile and execution can be timed)."""
    if "runner" in _NC_CACHE:
        return _NC_CACHE["runner"]
    import jax
    import jax.numpy as jnp  # noqa: F401
    from jax.experimental.shard_map import shard_map
    from jax.sharding import Mesh, PartitionSpec

    from concourse import mybir as _mb
    from concourse.bass2jax import (
        _bass_exec_p,
        install_neuronx_cc_hook,
        partition_id_tensor,
    )

    install_neuronx_cc_hook()
    nc = _get_nc()
    partition_name = (
        nc.partition_id_tensor.name if nc.partition_id_tensor else None
    )
    in_names, out_names, out_avals, zero_outs = [], [], [], []
    for alloc in nc.m.functions[0].allocations:
        if not isinstance(alloc, _mb.MemoryLocationSet):
            continue
        name = alloc.memorylocations[0].name
        if alloc.kind == "ExternalInput":
            if name != partition_name:
                in_names.append(name)
        elif alloc.kind == "ExternalOutput":
            out_names.append(name)
            shape = tuple(alloc.tensor_shape)
            dtype = _mb.dt.np(alloc.dtype)
            out_avals.append(jax.core.ShapedArray(shape, dtype))
            zero_outs.append(np.zeros(shape, dtype))
    n_params = len(in_names)
    all_in_names = list(in_names) + list(out_names)
    if partition_name is not None:
        all_in_names.append(partition_name)

    def _body(*args):
        operands = list(args)
        if partition_name is not None:
            operands.append(partition_id_tensor())
        outs = _bass_exec_p.bind(
            *operands,
            out_avals=tuple(out_avals),
            in_names=tuple(all_in_names),
            out_names=tuple(out_names),
            lowering_input_output_aliases=(),
            sim_require_finite=True,
            sim_require_nnan=True,
            nc=nc,
        )
        return tuple(outs)

    devices = jax.devices()[:N_CORES]
    mesh = Mesh(np.asarray(devices), ("core",))
    n_outs = len(out_names)
    in_specs = (PartitionSpec("core"),) * (n_params + n_outs)
    out_specs = (PartitionSpec("core"),) * n_outs
    # No donation: the kernel writes every element of its outputs, so the
    # zero "output operand" buffers can be reused across timed calls.
    sharded = jax.jit(
        shard_map(
            _body, mesh=mesh, in_specs=in_specs, out_specs=out_specs,
            check_rep=False,
        ),
        keep_unused=True,
    )
    runner = dict(
        jax=jax,
        fn=sharded,
        in_names=in_names,
        out_names=out_names,
        out_avals=out_avals,
        zero_outs=zero_outs,
        mesh=mesh,
    )
    _NC_CACHE["runner"] = runner
    return runner


def _run_spmd(in_maps, bench_iters=0):
    """Execute on 8 cores; returns (per-core outputs, exec_ns or None)."""
    global LAST_EXEC_NS
    r = _get_runner()
    jax = r["jax"]
    concat_in = [
        np.concatenate([np.asarray(m[name]) for m in in_maps], axis=0)
        for name in r["in_names"]
    ]
    concat_zeros = [
        np.zeros((N_CORES * z.shape[0], *z.shape[1:]), z.dtype)
        for z in r["zero_outs"]
    ]
    args = [jax.device_put(a) for a in concat_in + concat_zeros]
    for a in args:
        a.block_until_ready()

    out_arrs = r["fn"](*args)
    for o in out_arrs:
        o.block_until_ready()

    exec_ns = None
    if bench_iters:
        import time as _t

        # steady-state: issue bench_iters calls back-to-back, block at end
        t0 = _t.perf_counter()
        outs = None
        for _ in range(bench_iters):
            outs = r["fn"](*args)
        for o in outs:
            o.block_until_ready()
        t1 = _t.perf_counter()
        exec_ns = int((t1 - t0) / bench_iters * 1e9)
        LAST_EXEC_NS = exec_ns

    results = []
    for c in range(N_CORES):
        results.append(
            {
                name: np.asarray(out_arrs[i]).reshape(
                    N_CORES, *r["out_avals"][i].shape
                )[c]
                for i, name in enumerate(r["out_names"])
            }
        )
    return results, exec_ns


def _rope_tables(S_):
    half = D // 2
    freq = ROPE_BASE ** (-np.arange(half, dtype=np.float64) * 2.0 / D)
    ang = np.arange(S_, dtype=np.float64)[:, None] * freq[None, :]
    return (
        np.cos(ang).astype(np.float32),
        np.sin(ang).astype(np.float32),
    )


def kernel(x, Wq, Wk, Wv, Wo):
    global LAST_RESULTS
    x = np.asarray(x, dtype=np.float32)
    Wq = np.asarray(Wq, dtype=np.float32)
    Wk = np.asarray(Wk, dtype=np.float32)
    Wv = np.asarray(Wv, dtype=np.float32)
    Wo = np.asarray(Wo, dtype=np.float32)

    cosb, sinb = _rope_tables(S)

    xTs = [np.ascontiguousarray(x[b].astype(_BF16).T) for b in range(B)]
    wqs, wks, wvs, wos = [], [], [], []
    for g in range(GROUPS):
        q0, k0 = g * HQL, g * HKVL
        wqs.append(
            np.ascontiguousarray(
                Wq[:, q0 : q0 + HQL].reshape(E, HQL * D).astype(_BF16)
            )
        )
        wks.append(
            np.ascontiguousarray(
                Wk[:, k0 : k0 + HKVL].reshape(E, HKVL * D).astype(_BF16)
            )
        )
        wvs.append(
            np.ascontiguousarray(
                Wv[:, k0 : k0 + HKVL].reshape(E, HKVL * D).astype(_BF16)
            )
        )
        wos.append(
            np.ascontiguousarray(
                Wo[q0 : q0 + HQL].reshape(HQL * D, E).astype(_BF16)
            )
        )

    in_maps = []
    for core in range(N_CORES):
        b, g = core // GROUPS, core % GROUPS
        in_maps.append(
            {
                "xT": xTs[b],
                "wq": wqs[g],
                "wk": wks[g],
                "wv": wvs[g],
                "wo": wos[g],
                "cosb": cosb,
                "sinb": sinb,
            }
        )

    bench_iters = int(os.environ.get("KERNEL_BENCH_ITERS", "0"))
    results, _ = _run_spmd(in_maps, bench_iters=bench_iters)
    LAST_RESULTS = results

    out = np.zeros((B, S, E), dtype=np.float32)
    for core in range(N_CORES):
        b = core // GROUPS
        out[b] += results[core]["out"].astype(np.float32)
    return out
